# revision 90
# baseline (speedup 1.0000x reference)
"""Cross-attention (B=2, N=M=2048, DIM=1024, H=16) on 8 TRN2 NeuronCores.

Sharding: tensor-parallel over heads. Core i owns heads 2i,2i+1 (128 of the
1024 q/k/v dims). Attention runs in two head-halves (A = head 2i, B = head
2i+1): half A for all 8 token chunks, then half B, so the half-A AllToAll
flies while half B computes. Output projection = half-A part (woven into
phase C_B) + half-B part (tail).

Scheduling: PE p-state needs ~3us of gap-free execution to reach 2.4 GHz,
so the emission is software-pipelined at slot granularity: unit k's score
slots carry unit k-1's AV matmuls plus ~1 weave quantum (projection
matmuls) sized to slightly overfill the ACT exp cadence (~1.06us/tile).
The PE then never reaches a semaphore wait before it is satisfied.

Compute dtype: bf16 matmul operands, f32 PSUM accumulation.
"""

import sys

for _p in ("/opt/trn_rl_repo",):
    if _p not in sys.path:
        sys.path.append(_p)

from collections import deque

import ml_dtypes
import numpy as np

import concourse.mybir as mybir
import concourse.tile as tile
from concourse import bacc

NCORES = 8
B, N, M, DIM, H = 2, 2048, 2048, 1024, 16
D = DIM // H                  # 64 head dim
HPC = H // NCORES             # 2 heads per core
DLOC = HPC * D                # 128 local q/k/v dims per core
TOK = B * N                   # 4096 query tokens (flattened b-major)
MTOK = B * M                  # 4096 key tokens
TSL = TOK // NCORES           # 512-token output slice per core
SCALE = float(D) ** -0.5

KT = DIM // 128               # 8 contraction tiles for projections
NB = 512                      # matmul moving free dim / psum bank width
MT = M // 128                 # 16 m-tiles per batch
NCH = TOK // NB               # 8 token chunks of 512
NU = 16                       # units = (half, b, qb)
JP = MT // 2                  # 8 mt-pairs (slots) per unit

BF16 = mybir.dt.bfloat16
F32 = mybir.dt.float32
AF = mybir.ActivationFunctionType

# estimated PE cycles per slot @2.4GHz: scores pair (1024) + av pair (1024)
# vs ACT cadence ~1.06us = ~2550 cyc; overfill to ~2850.
SLOT_TARGET = 2850


def build():
    nc = bacc.Bacc("TRN2", target_bir_lowering=False, debug=False,
                   num_devices=NCORES)

    x1t = nc.declare_dram_parameter("x1t", [TOK // NB, 128, KT, NB], BF16,
                                    isOutput=False)
    x2t = nc.declare_dram_parameter("x2t", [MTOK // NB, 128, KT, NB], BF16,
                                    isOutput=False)
    wq = nc.declare_dram_parameter("wq", [KT, 128, DLOC], BF16, isOutput=False)
    wk = nc.declare_dram_parameter("wk", [KT, 128, DLOC], BF16, isOutput=False)
    wv = nc.declare_dram_parameter("wv", [KT, 128, DLOC], BF16, isOutput=False)
    wpa = nc.declare_dram_parameter("wpa", [KT // 2, 128, DIM], BF16,
                                    isOutput=False)
    wpb = nc.declare_dram_parameter("wpb", [KT // 2, 128, DIM], BF16,
                                    isOutput=False)
    bp = nc.declare_dram_parameter("bp", [1, DIM], F32, isOutput=False)
    out = nc.declare_dram_parameter("out", [TSL, DIM], F32, isOutput=True)

    ata_in = [nc.dram_tensor(f"ata_in{h}", [NCORES, D, TSL], BF16)
              for h in range(2)]
    ata_out = [nc.dram_tensor(f"ata_out{h}", [NCORES, D, TSL], BF16)
               for h in range(2)]

    with tile.TileContext(nc) as tc:
        with (
            tc.tile_pool(name="persist", bufs=1) as pp,
            tc.tile_pool(name="xin", bufs=8) as xp,
            tc.tile_pool(name="work", bufs=3) as wkp,
            tc.tile_pool(name="norm", bufs=2) as np_,
            tc.tile_pool(name="yout", bufs=2) as yp,
        ):
            wq_sb = pp.tile([128, KT, DLOC], BF16, tag="wq")
            wk_sb = pp.tile([128, KT, DLOC], BF16, tag="wk")
            wv_sb = pp.tile([128, KT, DLOC], BF16, tag="wv")
            wp_sb = [pp.tile([128, KT // 2, DIM], BF16, tag=f"wp{h}",
                             name=f"wp{h}")
                     for h in range(2)]
            bp_sb = pp.tile([1, DIM], F32, tag="bp")
            bias_bc = pp.tile([128, DIM], F32, tag="bias_bc")
            qt_b = [pp.tile([128, N], BF16, tag=f"qt{b}", name=f"qt{b}")
                    for b in range(B)]
            kt_b = [pp.tile([128, M], BF16, tag=f"kt{b}", name=f"kt{b}")
                    for b in range(B)]
            v_b = [pp.tile([128, M // 128, HPC, D + 1], BF16, tag=f"v{b}",
                           name=f"v{b}")
                   for b in range(B)]
            ot_sb = pp.tile([128, NCH, NB], BF16, tag="ot")
            of_sb = [pp.tile([128, KT // 2, TSL], BF16, tag=f"of{h}",
                             name=f"of{h}")
                     for h in range(2)]
            ya_sb = pp.tile([128, TSL // 128, DIM], F32, tag="ya")

            # k/q/v weights on gpsimd (interleaved with the first x2
            # fetches, issued by the prologue below); wp/bias on the idle
            # scalar DGE queue (not needed until ~unit 13)
            # small k/q/v weights on the fast scalar HWDGE queue (32KB
            # each, ~0.5us): the first k-proj group needs ALL wk chunks,
            # and gpsimd's software DGE takes ~1-2us per descriptor
            # each dma_start costs ~0.6-1us of sequencer issue time
            # (DIRECT2D), so weights load as single rearranged DMAs
            def load_w_kq():
                nc.scalar.dma_start(wk_sb[:],
                                    wk[:].rearrange("k p d -> p k d"))
                nc.scalar.dma_start(wq_sb[:, 0, :], wq[0])

            def load_w_rest():
                nc.sync.dma_start(wv_sb[:],
                                  wv[:].rearrange("k p d -> p k d"))
                nc.sync.dma_start(wq_sb[:, 1:KT, :],
                                  wq[1:KT].rearrange("k p d -> p k d"))
                nc.sync.dma_start(wp_sb[0][:],
                                  wpa[:].rearrange("k p d -> p k d"))
                nc.sync.dma_start(wp_sb[1][:],
                                  wpb[:].rearrange("k p d -> p k d"))
                nc.sync.dma_start(bp_sb[:], bp[:])
                nc.gpsimd.partition_broadcast(bias_bc[:], bp_sb[0:1, :])

            for b in range(B):
                nc.vector.memset(v_b[b][:, :, :, D], 1.0)

            x_tiles = {}

            def fetch_x(which, b, nb, eng=None):
                t = xp.tile([128, KT, NB], BF16, tag=f"x{which}", bufs=4,
                            name=f"x{which}_{b}{nb}")
                src = x1t if which == 1 else x2t
                (eng or nc.sync).dma_start(t[:], src[(N // NB) * b + nb])
                x_tiles[(which, b, nb)] = t

            with (
                tc.tile_pool(name="ps_s", bufs=2, space="PSUM") as pss,
                tc.tile_pool(name="ps_o", bufs=2, space="PSUM") as pso,
                tc.tile_pool(name="ps_b", bufs=2, space="PSUM") as psb,
            ):
                # HAM-warming dummy matmuls for the AllToAll#2 tail
                # window only — they recycle the by-then-idle scores pool
                # (zero-dep reads of kt/qt, result never read)
                def dummy_mm():
                    t = pss.tile([128, 2 * NB], F32, tag="sps",
                                 name="dummy_ps")
                    for j2 in range(2):
                        nc.tensor.matmul(t[:, NB * j2:NB * (j2 + 1)],
                                         kt_b[0][0:64, 0:128],
                                         qt_b[0][0:64, 0:NB],
                                         start=True, stop=True)
                # ---------- weave quanta ----------
                # each group -> list of (est_cycles, fn); group psum tile is
                # created by the first quantum (shared via cell).

                def g_kq(kind, b, nb, cpeng=None):
                    # psum evacuation on ACT by default (idle capacity in
                    # the PE-bound phase); prologue groups use DVE so the
                    # first exps are not stuck behind the scalar queue's
                    # blocking dma_starts
                    w_sb = wk_sb if kind == "k" else wq_sb
                    dst = kt_b[b] if kind == "k" else qt_b[b]
                    xw, xb = (2, b) if kind == "k" else (1, b)
                    cell = {}
                    quanta = []
                    for k0 in range(0, KT, 2):
                        def fn(k0=k0):
                            if k0 == 0:
                                cell["t"] = psb.tile([128, NB], F32,
                                                     tag="bps", name="kq_ps")
                            t = cell["t"]
                            xt = x_tiles[(xw, xb, nb)]
                            for k in (k0, k0 + 1):
                                nc.tensor.matmul(t[:], w_sb[:, k, :],
                                                 xt[:, k, :],
                                                 start=(k == 0),
                                                 stop=(k == KT - 1))
                            if k0 + 2 == KT:
                                if cpeng is nc.vector:
                                    nc.vector.tensor_copy(
                                        dst[:, NB * nb:NB * (nb + 1)],
                                        t[:])
                                else:
                                    nc.scalar.copy(
                                        dst[:, NB * nb:NB * (nb + 1)],
                                        t[:])
                        quanta.append((1024, fn))
                    return quanta

                def g_v(b, nb):
                    quanta = []
                    for j in range(NB // 128):
                        def fn(j=j):
                            xt = x_tiles[(2, b, nb)]
                            mc = nb * (NB // 128) + j
                            v_ps = psb.tile([128, NB], F32, tag="bps",
                                            name="v_ps")
                            for k in range(KT):
                                nc.tensor.matmul(
                                    v_ps[:, 0:DLOC],
                                    xt[:, k, 128 * j:128 * (j + 1)],
                                    wv_sb[:, k, :],
                                    start=(k == 0), stop=(k == KT - 1))
                            for hh in range(HPC):
                                nc.scalar.copy(
                                    v_b[b][:, mc, hh, 0:D],
                                    v_ps[:, D * hh:D * (hh + 1)])
                        quanta.append((1100, fn))
                    return quanta

                def g_proj(half, tt, eb, pool=None, ptag=None):
                    cell = {}
                    quanta = []
                    for k0 in range(0, KT // 2, 2):
                        def fn(k0=k0):
                            tsl_ = slice(128 * tt, 128 * (tt + 1))
                            esl = slice(NB * eb, NB * (eb + 1))
                            if k0 == 0:
                                cell["t"] = (pool or psb).tile(
                                    [128, NB], F32, tag=ptag or "bps",
                                    name="y_ps")
                            t = cell["t"]
                            for k in (k0, k0 + 1):
                                nc.tensor.matmul(t[:],
                                                 of_sb[half][:, k, tsl_],
                                                 wp_sb[half][:, k, esl],
                                                 start=(k == 0),
                                                 stop=(k == KT // 2 - 1))
                            if k0 + 2 == KT // 2:
                                if half == 0:
                                    nc.vector.tensor_add(
                                        ya_sb[:, tt, esl], t[:],
                                        bias_bc[:, esl])
                                else:
                                    y_sb = yp.tile([128, NB], F32,
                                                   tag="ysb")
                                    nc.vector.tensor_add(
                                        y_sb[:], t[:], ya_sb[:, tt, esl])
                                    # alternate output queues to halve
                                    # the final DMA drain
                                    oeng = (nc.sync if (2 * tt + eb) % 2
                                            else nc.scalar)
                                    oeng.dma_start(out[tsl_, esl],
                                                   y_sb[:])
                        quanta.append((1024, fn))
                    return quanta

                # ---------- due-ordered weave schedule ----------
                # units: 0..7 = half A (b0 q0..3, b1 q0..3), 8..15 = half B.
                # due = (unit, slot) BEFORE which the quantum must be done.
                # Emission order is semantic order, so the schedule is
                # stable-sorted by due before use.
                sched_items = []

                def add(due, quanta):
                    for q in quanta:
                        sched_items.append((due, len(sched_items), q))

                def add_fetch(due, which, b, nb):
                    add(due, [(0, lambda: fetch_x(which, b, nb))])

                # prologue (emitted directly): k(0,0) k(0,1) q(0,0)
                # k(b,nb) due (first unit of b, slot 2nb);
                # q(b,qb) due (unit of (b,qb), 0);
                # v(b,nb) due (unit after first unit of b, slot 2nb)
                add((0, 4), g_kq("k", 0, 2))
                add((0, 6), g_kq("k", 0, 3))
                add((1, 0), g_kq("q", 0, 1))
                add((1, 0), g_v(0, 0))
                add((1, 2), g_v(0, 1))
                add((1, 4), g_v(0, 2))
                add((1, 6), g_v(0, 3))
                add((2, 0), g_kq("q", 0, 2))
                # b1 x tiles fetched just-in-time: the fetch recycles a b0
                # x buffer, so it must be emitted after that buffer's last
                # reader (the b0 k/q/v quanta above).
                add_fetch((2, 0), 2, 1, 0)
                add_fetch((2, 4), 2, 1, 1)
                add((3, 0), g_kq("q", 0, 3))
                add_fetch((3, 0), 2, 1, 2)
                add_fetch((3, 2), 1, 1, 0)
                add_fetch((3, 4), 2, 1, 3)
                add_fetch((3, 6), 1, 1, 1)
                add((4, 0), g_kq("k", 1, 0))
                add((4, 0), g_kq("q", 1, 0))
                add((4, 2), g_kq("k", 1, 1))
                add_fetch((4, 2), 1, 1, 2)
                add((4, 4), g_kq("k", 1, 2))
                add((4, 6), g_kq("k", 1, 3))
                add_fetch((4, 6), 1, 1, 3)
                add((5, 0), g_v(1, 0))
                add((5, 0), g_kq("q", 1, 1))
                add((5, 2), g_v(1, 1))
                add((5, 4), g_v(1, 2))
                add((5, 6), g_v(1, 3))
                add((6, 0), g_kq("q", 1, 2))
                add((7, 0), g_kq("q", 1, 3))
                sched = deque(x for x in sorted(sched_items))
                woven_proj = [0]
                # half-A projection: available once AllToAll#1 landed
                # (trigger after unit 8, ~21us = ~2.5 units) -> woven from
                # unit 12; leftovers pad the AllToAll#2 window.
                proj_a = deque()
                for tt in range(TSL // 128):
                    for eb in range(DIM // NB):
                        proj_a.append(g_proj(0, tt, eb))

                def weave(u, jp, slot_cyc):
                    # force everything due before this slot
                    while sched and sched[0][0] <= (u, jp):
                        _, _, (cyc, fn) = sched.popleft()
                        fn()
                        slot_cyc += cyc
                    # fill to target with due-later work, then proj_A
                    # (gated past the collective#1 skew window; half kept
                    # back for the AllToAll#2 window)
                    while slot_cyc < SLOT_TARGET:
                        if sched:
                            _, _, (cyc, fn) = sched.popleft()
                        else:
                            break
                        fn()
                        slot_cyc += cyc
                    # units 9-15 have no real weave left: one dummy per
                    # slot (idle bps bank) keeps the HAM gate at 8/8.
                    # (proj_A must NOT weave here: collective#1 duration
                    # varies 15-45us with cross-core skew and a too-early
                    # of_A read head-blocks the whole PE queue.)
                    if slot_cyc < 2550 and 9 <= u <= 15:
                        t = psb.tile([128, NB], F32, tag="bps",
                                     name="dummy_ps")
                        nc.tensor.matmul(t[:], kt_b[0][0:64, 0:128],
                                         qt_b[0][0:64, 0:NB],
                                         start=True, stop=True)


                units = [(h, b, qb) for h in range(2) for b in range(B)
                         for qb in range(N // NB)]

                pts_prev = None
                unit_prev = None
                o_prev = None

                def emit_av_pair(uprev, pts, jp):
                    half, b, qb = uprev
                    for j2 in range(2):
                        mt = 2 * jp + j2
                        nc.tensor.matmul(
                            o_prev[0:D + 1, :],
                            v_b[b][:, mt, half, :],
                            pts[jp][:, NB * j2:NB * (j2 + 1)],
                            start=(mt == 0), stop=(mt == MT - 1))

                fixups = []

                def emit_normalize(uprev, shadow=False, fast=False):
                    # gpsimd partition_broadcast of 1/denominator. In the
                    # collective#1 shadow gpsimd is blocked, so those
                    # units store the UNNORMALIZED output (freeing o_ps
                    # for the AV pipeline) plus the recip, and are fixed
                    # up in place once gpsimd unblocks. Shadow chunks are
                    # half-B, consumed only by AllToAll#2 much later.
                    half, b, qb = uprev
                    ch = (N * b) // NB + qb
                    hsl = slice(D * half, D * (half + 1))
                    rc = np_.tile([1, NB], F32, tag="recipf", bufs=5,
                                  name="rc")
                    if fast:
                        # last unit: recip gates the AllToAll#2 trigger;
                        # approx_fast (18 bits) needs an SBUF f32 input
                        dcp = np_.tile([1, NB], F32, tag="dcp", bufs=1,
                                       name="dcp")
                        nc.vector.tensor_copy(dcp[:], o_prev[D:D + 1, :])
                        nc.vector.reciprocal_approx_fast(rc[:], dcp[:])
                    else:
                        nc.vector.reciprocal(rc[:], o_prev[D:D + 1, :])
                    if shadow:
                        nc.vector.tensor_copy(ot_sb[hsl, ch, :],
                                              o_prev[0:D, :])
                        fixups.append((rc, hsl, ch))
                    else:
                        bc = np_.tile([D, NB], F32, tag="bcast",
                                      name="bc")
                        nc.gpsimd.partition_broadcast(bc[:], rc[0:1, :])
                        nc.vector.tensor_mul(ot_sb[hsl, ch, :],
                                             o_prev[0:D, :], bc[:])

                def flush_fixups():
                    # both SB operands of tensor_mul must share a base
                    # partition, so broadcast into the matching rows of a
                    # full-height tile
                    while fixups:
                        rc, hsl, ch = fixups.pop(0)
                        bc = np_.tile([128, NB], F32, tag="bcfix",
                                      bufs=1, name="bc")
                        nc.gpsimd.partition_broadcast(bc[:], rc[0:1, :])
                        nc.vector.tensor_mul(ot_sb[hsl, ch, :],
                                             ot_sb[hsl, ch, :],
                                             bc[hsl, :])

                def emit_ata_early(half):
                    # stage chunks 0-6 before the last chunk's normalize
                    # chain (~5us of recip+broadcast) completes
                    rsl = slice(D * half, D * (half + 1))
                    nc.sync.dma_start(
                        ata_in[half][0:NCH - 1].rearrange("c p t -> p c t"),
                        ot_sb[rsl, 0:NCH - 1, :])

                def emit_collective(half):
                    rsl = slice(D * half, D * (half + 1))
                    nc.sync.dma_start(ata_in[half][NCH - 1],
                                      ot_sb[rsl, NCH - 1, :])
                    nc.gpsimd.collective_compute(
                        "AllToAll", mybir.AluOpType.bypass,
                        replica_groups=[list(range(NCORES))],
                        ins=[ata_in[half].ap().opt()],
                        outs=[ata_out[half].ap().opt()],
                    )
                    for k in range(KT // 2):
                        nc.sync.dma_start(of_sb[half][0:D, k, :],
                                          ata_out[half][2 * k])
                        nc.sync.dma_start(of_sb[half][D:128, k, :],
                                          ata_out[half][2 * k + 1])

                # ---------- prologue (b0 x tiles only) ----------
                # wk first (tiny, the first k-proj needs all 8 chunks),
                # then x2 on the same scalar HWDGE queue; x1 on sync
                fetch_x(2, 0, 0, nc.scalar)
                load_w_kq()
                fetch_x(2, 0, 1, nc.scalar)
                fetch_x(1, 0, 0)
                fetch_x(2, 0, 2)
                fetch_x(2, 0, 3)
                load_w_rest()
                for nb in range(1, 4):
                    fetch_x(1, 0, nb)
                for _, fn in g_kq("k", 0, 0) + g_kq("k", 0, 1) + \
                        g_kq("q", 0, 0):
                    fn()

                # ---------- pipelined units ----------
                for ui, unit in enumerate(units):
                    half, b, qb = unit
                    hsl = slice(D * half, D * (half + 1))
                    lnsl = slice(NB * qb, NB * (qb + 1))
                    o_cur = pso.tile([128, NB], F32, tag="ops", name="o_ps")
                    if ui == 14:
                        flush_fixups()
                    pts = []
                    for jp in range(JP):
                        weave(ui, jp, 2048 if pts_prev is not None else 1024)
                        s_ps = pss.tile([128, 2 * NB], F32, tag="sps",
                                        name="s_ps")
                        for j2 in range(2):
                            mt = 2 * jp + j2
                            msl = slice(128 * mt, 128 * (mt + 1))
                            nc.tensor.matmul(
                                s_ps[:, NB * j2:NB * (j2 + 1)],
                                kt_b[b][hsl, msl],
                                qt_b[b][hsl, lnsl],
                                start=True, stop=True)
                        pt = wkp.tile([128, 2 * NB], BF16, tag="pt",
                                      bufs=17, name="pt")
                        nc.scalar.activation(pt[:], s_ps[:], AF.Exp,
                                             scale=SCALE)
                        pts.append(pt)
                        if pts_prev is not None:
                            emit_av_pair(unit_prev, pts_prev, jp)
                    if pts_prev is not None:
                        emit_normalize(unit_prev, shadow=(ui in (9, 10, 11, 12)))
                    pts_prev, unit_prev, o_prev = pts, unit, o_cur
                    if ui == 7:
                        # chunks 0-6 of half A are normalized (unit 6's
                        # normalize ran during unit 7)
                        emit_ata_early(0)
                    if ui == 8:
                        # last A-unit (7) was normalized during unit 8
                        emit_collective(0)

                # drain: AV + normalize of the last unit; half-B chunks
                # 0-6 stage while the drain runs
                emit_ata_early(1)
                for jp in range(JP):
                    emit_av_pair(unit_prev, pts_prev, jp)
                    if jp >= 6 and proj_a:
                        # exp of the last pairs lags the drain burst; fill
                        # with a real proj quantum instead of stalling
                        cyc, fn = proj_a[0].pop(0)
                        if not proj_a[0]:
                            proj_a.popleft()
                        fn()
                emit_normalize(unit_prev, fast=True)
                emit_collective(1)

                # A2A#2 window: leftover half-A projection, then dummies
                # sized to the collective latency keep HAM warm
                while sched:
                    _, _, (cyc, fn) = sched.popleft()
                    fn()
                while proj_a:
                    for cyc, fn in proj_a.popleft():
                        fn()
                for _ in range(20):
                    dummy_mm()
                gi = 0

                # tail: half-B projection + output; groups alternate PSUM
                # pools and a dummy masks the DVE-add latency
                for tt in range(TSL // 128):
                    for eb in range(DIM // NB):
                        pool, ptag = ((psb, "bps") if gi % 2 == 0
                                      else (pso, "ops"))
                        for cyc, fn in g_proj(1, tt, eb, pool, ptag):
                            fn()
                        gi += 1

    nc.compile()
    return nc


def _tile_xt(x):
    """[B,N,DIM] f32 -> [TOK//NB, 128, KT, NB] bf16 block-contiguous x^T."""
    bf = ml_dtypes.bfloat16
    xt = x.reshape(TOK, DIM).T
    return np.ascontiguousarray(
        xt.reshape(KT, 128, TOK // NB, NB).transpose(2, 1, 0, 3)).astype(bf)


def make_in_maps(x1, x2, Wq, Wkv, Wproj, bproj):
    bf = ml_dtypes.bfloat16
    x1t = _tile_xt(x1)
    x2t = _tile_xt(x2)
    wk_full = Wkv[:, :DIM]
    wv_full = Wkv[:, DIM:]
    wpr = Wproj.reshape(NCORES, 2, D, DIM)
    wpa = np.ascontiguousarray(
        wpr[:, 0].reshape(KT // 2, 128, DIM)).astype(bf)
    wpb = np.ascontiguousarray(
        wpr[:, 1].reshape(KT // 2, 128, DIM)).astype(bf)
    bp = bproj.reshape(1, DIM).astype(np.float32)
    in_maps = []
    for c in range(NCORES):
        sl = slice(DLOC * c, DLOC * (c + 1))
        in_maps.append({
            "x1t": x1t, "x2t": x2t,
            "wq": np.ascontiguousarray(Wq[:, sl]).reshape(KT, 128, DLOC).astype(bf),
            "wk": np.ascontiguousarray(wk_full[:, sl]).reshape(KT, 128, DLOC).astype(bf),
            "wv": np.ascontiguousarray(wv_full[:, sl]).reshape(KT, 128, DLOC).astype(bf),
            "wpa": wpa, "wpb": wpb, "bp": bp,
        })
    return in_maps


_nc = None


def run(inputs, trace=False):
    global _nc
    from concourse.bass_utils import run_bass_kernel_spmd
    if _nc is None:
        _nc = build()
    in_maps = make_in_maps(**inputs)
    res = run_bass_kernel_spmd(_nc, in_maps, core_ids=list(range(NCORES)),
                               trace=trace)
    y = np.concatenate([res.results[c]["out"] for c in range(NCORES)], axis=0)
    return y.reshape(B, N, DIM), res


def kernel(x1, x2, Wq, Wkv, Wproj, bproj):
    y, _ = run(dict(x1=x1, x2=x2, Wq=Wq, Wkv=Wkv, Wproj=Wproj, bproj=bproj))
    return y
